# revision 1
# baseline (speedup 1.0000x reference)
"""Trainium2 Bass kernel for the MatchMatrix pairwise MLP.

kernel(**inputs) takes the FULL inputs (sent_a/sent_b [8,128,256], W1 [512,64],
b1, W2, b2, W3, b3) and returns the FULL [8,128,128,16] float32 output.

Sharding: data-parallel over batch B=8 -> one batch element per NeuronCore
(8 cores); weights/biases are replicated. Each core runs a fused Bass/Tile
kernel (built below) and the per-core [128,128,16] outputs are stacked.

Per-core layout (k-on-partitions):
  j = 16t + 4jj + 2u + e  (t 0..7, jj 0..3 free blocks, u group parity, e half)
  L1: h1a_g [128=(e,k0), 512=(jj,a)] = leaky(a_part + b_part + b1), computed
      as broadcast add + leaky split across ACT (bias-fused), DVE and Pool,
      software-pipelined 3 stages ahead of L2.
  L2: psum2_t [128=(u,e,k1), 512] = blockdiag(W2,W2).T @ h1a (2 mms/t)
  L3 fused with output transpose: psO[128=a, 64=(c,k2)] = h2_block.T @ W3dd
      (+ K=1 ones x b3 row matmul for the bias), leaky -> SBUF -> DMA,
      trailing L2 by 2 stages.
Matmul operands are fp16 (1 cyc/row on PE, ~7e-4 rel err end to end);
all accumulation stays fp32 in PSUM. Dummy matmuls during the DMA window
pre-warm the tensor-engine clock (HAM).
"""
import sys
import numpy as np

for _p in ("/opt/trn_rl_repo", "/root/.axon_site/_ro/trn_rl_repo"):
    if _p not in sys.path:
        sys.path.append(_p)

from contextlib import ExitStack

import concourse.bass as bass
import concourse.tile as tile
from concourse import bacc, mybir, masks
from concourse import bass_utils

F32 = mybir.dt.float32
BF16 = mybir.dt.bfloat16
FP16 = mybir.dt.float16
AF = mybir.ActivationFunctionType
ALU = mybir.AluOpType

def build_nc(act: str = "prelu", mm: str = "fp16", reps: int = 1,
             l1_split=(6, 6, 4), pe_warm: int = 5):
    nc = bacc.Bacc("TRN2", target_bir_lowering=False, debug=False, num_devices=8)
    sa = nc.dram_tensor("sent_a", [128, 256], F32, kind="ExternalInput").ap()
    sb = nc.dram_tensor("sent_b", [128, 256], F32, kind="ExternalInput").ap()
    W1 = nc.dram_tensor("W1", [512, 64], F32, kind="ExternalInput").ap()
    b1 = nc.dram_tensor("b1", [64], F32, kind="ExternalInput").ap()
    W2 = nc.dram_tensor("W2", [64, 32], F32, kind="ExternalInput").ap()
    b2 = nc.dram_tensor("b2", [32], F32, kind="ExternalInput").ap()
    W3 = nc.dram_tensor("W3", [32, 16], F32, kind="ExternalInput").ap()
    b3 = nc.dram_tensor("b3", [16], F32, kind="ExternalInput").ap()
    out = nc.dram_tensor("out", [128, 128, 16], F32, kind="ExternalOutput").ap()

    with tile.TileContext(nc) as tc, ExitStack() as ctx:
        _body(ctx, tc, sa, sb, W1, b1, W2, b2, W3, b3, out, act, mm, reps, l1_split, pe_warm)
    nc.compile()
    return nc


def _body(ctx, tc, sa, sb, W1, b1, W2, b2, W3, b3, out, act, mm, reps, l1_split, pe_warm):
    nc = tc.nc
    mm_dt = {"bf16": BF16, "fp16": FP16, "f32": F32}[mm]
    # leaky slope (0.0 -> plain relu, used for CoreSim parity checks)
    alpha = 0.01 if act == "prelu" else 0.0

    def act_leaky(out_ap, in_ap, bias):
        if act == "prelu":
            nc.scalar.activation(out_ap, in_ap, AF.Prelu, bias=bias, alpha=alpha)
        else:
            nc.scalar.activation(out_ap, in_ap, AF.Relu, bias=bias)

    def dve_leaky(out_ap, in_ap):
        # out = max(alpha * x, x)
        nc.vector.scalar_tensor_tensor(
            out=out_ap, in0=in_ap, scalar=alpha, in1=in_ap,
            op0=ALU.mult, op1=ALU.max)

    const = ctx.enter_context(tc.tile_pool(name="const", bufs=1))
    work = ctx.enter_context(tc.tile_pool(name="work", bufs=4))
    h1pool = ctx.enter_context(tc.tile_pool(name="h1p", bufs=16))
    psS = ctx.enter_context(tc.tile_pool(name="psS", bufs=2, space="PSUM"))
    psA = ctx.enter_context(tc.tile_pool(name="psA", bufs=3, space="PSUM"))
    psC = ctx.enter_context(tc.tile_pool(name="psC", bufs=3, space="PSUM"))

    # ---------- constants / weights ----------
    # ident first: it gates the input transposes (head critical path)
    ident = const.tile([128, 128], F32, tag="ident")
    masks.make_identity(nc, ident[:])

    # ACT table prewarm: touch the leaky table before real work needs it.
    warm2 = const.tile([1, 1], F32, tag="warm2")
    act_leaky(warm2[:], ident[0:1, 0:1], 0.0)

    # PE clock warm-up: dummy matmuls fill the otherwise-idle DMA window so
    # HAM ramps the tensor-engine clock before real work arrives.
    if pe_warm:
        dmy_l = const.tile([128, 128], mm_dt, tag="dmyl")
        nc.vector.tensor_scalar_mul(dmy_l[:], ident[:], 0.0)
        dmy_r = const.tile([128, 512], mm_dt, tag="dmyr")
        nc.vector.tensor_scalar_mul(
            dmy_r[:].rearrange("p (r k) -> p r k", r=4),
            ident[:].unsqueeze(1).broadcast_to([128, 4, 128]), 0.0)
        for _w in range(pe_warm):
            dps = psA.tile([128, 512], F32, tag="ps2")
            nc.tensor.matmul(dps[:], dmy_l[:], dmy_r[:], start=True, stop=True)

    # inputs first (they gate the transposes) on separate queues
    sa_sb = const.tile([128, 256], F32, tag="sa")
    nc.sync.dma_start(sa_sb[:], sa[:])
    sb_sb = const.tile([128, 256], F32, tag="sb")
    nc.gpsimd.dma_start(sb_sb[:], sb[:])

    # W1 in one shot: W1all[p, (c k)] = W1[128c + p, k],  c in 0..3
    W1v = W1.rearrange("(c p) k -> p c k", p=128)
    W1all = const.tile([128, 256], F32, tag="w1all")
    nc.sync.dma_start(
        W1all[:, 0:128].rearrange("p (c k) -> p c k", c=2), W1v[:, 0:2, :])
    nc.gpsimd.dma_start(
        W1all[:, 128:256].rearrange("p (c k) -> p c k", c=2), W1v[:, 2:4, :])

    # Wa_dup_c [128,128] = [W1all[:, c*64:(c+1)*64]] twice along free
    Wa_dup = []
    for c in (0, 1):
        w = const.tile([128, 128], mm_dt, tag=f"wadup{c}")
        srcv = W1all[:, 64 * c : 64 * c + 64].unsqueeze(1).broadcast_to([128, 2, 64])
        nc.vector.tensor_copy(w[:].rearrange("p (d k) -> p d k", d=2), srcv)
        Wa_dup.append(w)
    Wbh = const.tile([128, 128], mm_dt, tag="wbh")
    nc.vector.tensor_copy(Wbh[:], W1all[:, 128:256])
    Wb = [Wbh[:, 0:64], Wbh[:, 64:128]]

    # W2dd [128,64] = blockdiag(W2, W2); W3dd [128,64] = blockdiag(W3 x4)
    W2st = const.tile([64, 32], F32, tag="w2st")
    nc.sync.dma_start(W2st[:], W2[:])
    W3st = const.tile([32, 16], F32, tag="w3st")
    nc.gpsimd.dma_start(W3st[:], W3[:])
    W2dd = const.tile([128, 64], mm_dt, tag="w2dd")
    nc.vector.tensor_scalar_mul(W2dd[:], ident[:, 0:64], 0.0)
    nc.vector.tensor_copy(W2dd[0:64, 0:32], W2st[:])
    nc.vector.tensor_copy(W2dd[64:128, 32:64], W2st[:])
    W3dd = const.tile([128, 64], mm_dt, tag="w3dd")
    nc.vector.tensor_scalar_mul(W3dd[:], ident[:, 0:64], 0.0)
    for c in range(4):
        nc.vector.tensor_copy(W3dd[32 * c : 32 * c + 32, 16 * c : 16 * c + 16], W3st[:])

    # bias columns via K=1 outer products: bXd[p, 0] = bX[pattern(p)]
    ones1 = const.tile([1, 1], F32, tag="ones1")
    nc.vector.tensor_scalar(out=ones1[:], in0=ident[0:1, 0:1], scalar1=0.0,
                            scalar2=1.0, op0=ALU.mult, op1=ALU.add)
    b1r = const.tile([1, 64], F32, tag="b1r")
    nc.sync.dma_start(b1r[:], b1[:].unsqueeze(0))
    b2r = const.tile([1, 32], F32, tag="b2r")
    nc.gpsimd.dma_start(b2r[:], b2[:].unsqueeze(0))
    b3r = const.tile([1, 16], F32, tag="b3r")
    nc.sync.dma_start(b3r[:], b3[:].unsqueeze(0))

    def bias_col(row_ap, repeat, width, tag):
        # row_ap [1, w] -> column [repeat*w, 1] with the row repeated
        rep = const.tile([1, repeat * width], F32, tag=tag + "row")
        nc.vector.tensor_copy(
            rep[:].rearrange("o (r k) -> o r k", r=repeat),
            row_ap.unsqueeze(1).broadcast_to([1, repeat, width]))
        ps = psS.tile([128, 128], F32, tag="pst")
        nc.tensor.matmul(ps[0 : repeat * width, 0:1], rep[:], ones1[:],
                         start=True, stop=True)
        col = const.tile([repeat * width, 1], F32, tag=tag)
        nc.vector.tensor_copy(col[:], ps[0 : repeat * width, 0:1])
        return col

    b1d = bias_col(b1r[:], 2, 64, "b1d")    # [128,1]: (e,k0)
    b2q = bias_col(b2r[:], 4, 32, "b2q")    # [128,1]: (u,e,k1)

    # fused-L3 bias: ones_col [1,128] (K=1 lhsT), b3rep [1,512] = 8 x (c,k2) row
    ones_col = const.tile([1, 128], mm_dt, tag="onescol")
    nc.vector.tensor_scalar(out=ones_col[:], in0=ident[0:1, :], scalar1=0.0,
                            scalar2=1.0, op0=ALU.mult, op1=ALU.add)
    b3row64 = const.tile([1, 64], F32, tag="b3row64")
    nc.vector.tensor_copy(
        b3row64[:].rearrange("o (c k) -> o c k", c=4),
        b3r[:].unsqueeze(1).broadcast_to([1, 4, 16]))
    b3rep = const.tile([1, 512], mm_dt, tag="b3rep")
    nc.vector.tensor_copy(
        b3rep[:].rearrange("o (r w) -> o r w", r=8),
        b3row64[:].unsqueeze(1).broadcast_to([1, 8, 64]))

    out_flat = out.rearrange("a j k -> a (j k)")
    out_q = [nc.sync, nc.gpsimd, nc.sync, nc.scalar]

    for _rep in range(reps):
        # ---------- stage 1: transposes + a2 + b_pairs ----------
        saT, sbT = [], []
        for src, dstlist, nm in ((sa_sb, saT, "saT"), (sb_sb, sbT, "sbT")):
            for c in (0, 1):
                ps = psS.tile([128, 128], F32, tag="pst")
                nc.tensor.transpose(ps[:], src[:, 128 * c : 128 * (c + 1)], ident[:])
                t = work.tile([128, 128], mm_dt, tag=f"{nm}{c}")
                nc.vector.tensor_copy(t[:], ps[:])
                dstlist.append(t)

        ps_a2 = psS.tile([128, 128], F32, tag="pst")
        nc.tensor.matmul(ps_a2[:], Wa_dup[0][:], saT[0][:], start=True, stop=False)
        nc.tensor.matmul(ps_a2[:], Wa_dup[1][:], saT[1][:], start=False, stop=True)
        a2 = work.tile([128, 128], F32, tag="a2")
        nc.scalar.activation(a2[:], ps_a2[:], AF.Identity, bias=b1d[:, 0:1])

        ps_bT = psS.tile([64, 128], F32, tag="pst")
        nc.tensor.matmul(ps_bT[:], Wb[0], sbT[0][:], start=True, stop=False)
        nc.tensor.matmul(ps_bT[:], Wb[1], sbT[1][:], start=False, stop=True)
        b_pairs = work.tile([128, 64], F32, tag="bpairs")
        nc.vector.tensor_copy(b_pairs[0:64, :], ps_bT[:, 0:128:2])
        nc.vector.tensor_copy(b_pairs[64:128, :], ps_bT[:, 1:128:2])

        # ---------- stage 2: main pairwise loop ----------
        # L1: h1a_g = leaky(a2 + b_pairs[:, q(g,jj)]), engine per group from l1_split
        # L2: psum2_t[128=(u,e,k1),512] = W2dd.T @ h1a  (2 mms per t)
        # L3 fused with transpose: psO[128=a, 64=(c,k2)] per (t,jj):
        #     lhsT = h2[:, jj*128:+128], rhs = W3dd  -> M=a, N=(c,k2)
        #     + bias row matmul (K=1 ones x b3rep) resets each psO bank first.
        n_act, n_dve, n_pool = l1_split
        assert n_act + n_dve + n_pool == 16
        # alternate engines within each t (one ACT group + one DVE/Pool group)
        import itertools as _it
        acts = _it.chain(["act"] * n_act, _it.repeat(None))
        others = _it.chain(["dve"] * n_dve + ["pool"] * n_pool, _it.repeat(None))
        L1_MAP = []
        for _t in range(8):
            x = next(acts) or next(others)
            y = next(others) or next(acts)
            L1_MAP.extend([x, y])
        assert all(L1_MAP) and len(L1_MAP) == 16
        a2_view = a2[:].unsqueeze(1).broadcast_to([128, 4, 128])

        def l1_group(g, h1a):
            t_, u = g >> 1, g & 1
            q0 = 8 * t_ + u
            eng = L1_MAP[g]
            if eng == "act":
                for jj in range(4):
                    q = q0 + 2 * jj
                    act_leaky(h1a[:, 128 * jj : 128 * jj + 128], a2[:],
                              b_pairs[:, q : q + 1])
            else:
                h1 = h1pool.tile([128, 512], F32, tag="h1")
                bv = b_pairs[:, q0 : q0 + 7 : 2].unsqueeze(2).broadcast_to([128, 4, 128])
                hv = h1[:].rearrange("p (j a) -> p j a", j=4)
                e_ = nc.vector if eng == "dve" else nc.gpsimd
                e_.tensor_tensor(out=hv, in0=a2_view, in1=bv, op=ALU.add)
                # Pool lacks TensorScalarPtr on V3; its leaky runs on DVE
                nc.vector.scalar_tensor_tensor(
                    out=h1a[:], in0=h1[:], scalar=alpha, in1=h1[:],
                    op0=ALU.mult, op1=ALU.max)

        # software-pipelined: L1 runs 2 stages ahead of L2; L3' trails L2 by 1
        h1a_tiles = {}

        def make_l1(g):
            h1a = h1pool.tile([128, 512], mm_dt, tag="h1a")
            l1_group(g, h1a)
            h1a_tiles[g] = h1a

        def l3_stage(t_, h2):
            nonlocal psO
            if t_ % 2 == 0:
                psO = psC.tile([128, 512], F32, tag="psO")
                nc.tensor.matmul(psO[:], ones_col[:], b3rep[:],
                                 start=True, stop=False, skip_group_check=True)
            for jj in range(4):
                col = 256 * (t_ & 1) + 64 * jj
                last = (t_ % 2 == 1) and (jj == 3)
                nc.tensor.matmul(
                    psO[:, col : col + 64],
                    h2[:, 128 * jj : 128 * jj + 128],
                    W3dd[:],
                    start=False, stop=last, skip_group_check=True)
            if t_ % 2 == 1:
                s_ = t_ >> 1
                osb = work.tile([128, 512], F32, tag="osb")
                act_leaky(osb[:], psO[:], 0.0)
                out_q[s_].dma_start(out_flat[:, 512 * s_ : 512 * (s_ + 1)], osb[:])

        psO = None
        h2_tiles = {}
        for g in range(6):
            make_l1(g)
        dps_loop = None
        for t_ in range(8):
            ps2 = psA.tile([128, 512], F32, tag="ps2")
            for u in (0, 1):
                nc.tensor.matmul(
                    ps2[64 * u : 64 * u + 64, :], W2dd[:],
                    h1a_tiles.pop(2 * t_ + u)[:],
                    start=True, stop=True)
            for g in (2 * t_ + 6, 2 * t_ + 7):
                if g < 16:
                    make_l1(g)
            if pe_warm:
                dps = psS.tile([128, 512], F32, tag="pst")
                nc.tensor.matmul(dps[:], dmy_l[:], dmy_r[:], start=True, stop=True)
            h2 = work.tile([128, 512], mm_dt, tag="h2")
            act_leaky(h2[:], ps2[:], b2q[:, 0:1])
            h2_tiles[t_] = h2
            if t_ >= 2:
                l3_stage(t_ - 2, h2_tiles.pop(t_ - 2))
        l3_stage(6, h2_tiles.pop(6))
        l3_stage(7, h2_tiles.pop(7))


_NC_CACHE = {}


def _get_nc():
    if "nc" not in _NC_CACHE:
        _NC_CACHE["nc"] = build_nc()
    return _NC_CACHE["nc"]


def kernel(sent_a, sent_b, W1, b1, W2, b2, W3, b3):
    sent_a = np.ascontiguousarray(np.asarray(sent_a, dtype=np.float32))
    sent_b = np.ascontiguousarray(np.asarray(sent_b, dtype=np.float32))
    W1 = np.ascontiguousarray(np.asarray(W1, dtype=np.float32))
    b1 = np.ascontiguousarray(np.asarray(b1, dtype=np.float32))
    W2 = np.ascontiguousarray(np.asarray(W2, dtype=np.float32))
    b2 = np.ascontiguousarray(np.asarray(b2, dtype=np.float32))
    W3 = np.ascontiguousarray(np.asarray(W3, dtype=np.float32))
    b3 = np.ascontiguousarray(np.asarray(b3, dtype=np.float32))

    nc = _get_nc()
    in_maps = [{
        "sent_a": sent_a[i], "sent_b": sent_b[i],
        "W1": W1, "b1": b1, "W2": W2, "b2": b2, "W3": W3, "b3": b3,
    } for i in range(8)]
    res = bass_utils.run_bass_kernel_spmd(nc, in_maps, core_ids=list(range(8)))
    return np.stack([res.results[i]["out"] for i in range(8)]).astype(np.float32)

